# revision 38
# baseline (speedup 1.0000x reference)
"""Trainium2 Bass kernel for MinibatchDiscrimination.

Reference computation (B=256, IN=1024, O=64, K=50):
    M = (x @ T).reshape(B, O, K)
    l1[i,j,o] = sum_k |M[i,o,k] - M[j,o,k]|
    out = concat([x, sum_j exp(-l1) - 1], axis=1)          # [B, IN + O]

Sharding: the O (out_features) dimension is split across the 8 NeuronCores
(8 features per core); x is replicated. Each core computes its [256, 8]
feature block; the host gathers the blocks and concatenates with x.

Per-core pipeline:
  1. PE matmul: M[256, 400] = xT.T @ T_local (fp8 in, f32 PSUM), cast to
     fp8 -- the canonical value used on BOTH sides of the pairwise
     subtraction, so the diagonal distance is exactly zero.  +M is staged
     to DRAM as flat j-major rows.  Input DMAs are pipelined per
     contraction chunk so the matmul starts as soon as the first 1/8th
     of x and T arrive.
  2. All-pairs signed differences generated by the PE with an affine
     matmul: lhsT = [M_o^T (50 k-rows); -ones] and rhs = [I50 tiled over
     j; +M row].  Chunks of 32 j land in PSUM as [128, 4x512] f32.
  3. Symmetry: itile-1 blocks only compute j in [128,256); the mirrored
     contribution comes from PE column-sums of the itile-0 exp tiles.
  4. Consumers per 4-bank chunk, balanced across engines: bank 0 takes a
     fused DVE tensor_reduce(add, |.|) straight from PSUM into the l1
     slot; banks 1-3 take one wide ScalarE Abs -> bf16 SBUF, reduced
     50->1 by a dense DVE binary add-tree at bf16 2x rate (tree emitted
     immediately per 4-chunk group; interleaving or GpSimd offload both
     measured slower due to DVE FIFO ordering and SBUF-port contention).
  5. ScalarE exp(-l1) with accum_out producing the j-sum directly
     (no separate DVE reduction).
"""

import numpy as np
import ml_dtypes

B = 256
IN_FEATURES = 1024
O_TOTAL = 64
K = 50
KH = 25                             # k-pairs per feature
N_CORES = 8
O_LOC = O_TOTAL // N_CORES          # 8 features per core
N_LOC = O_LOC * K                   # 400 M' columns per core
P = 128                             # partitions
ITILES = B // P                     # 2 row tiles
CC = IN_FEATURES // P               # 8 contraction chunks
JCHUNK = 32                         # j's per PSUM chunk
JBANK = 8                           # j's per PSUM bank (8*50 = 400 of 512)
QB = JCHUNK // JBANK                # banks per chunk = 4
ABANK = 1                           # banks 0..ABANK-1 -> DVE direct-reduce
BQ = QB - ABANK                     # banks per chunk on the ScalarE path
NCHUNK = B // JCHUNK                # 8 chunks per full block
CPG = 4                             # chunks per tree group
NGROUP = NCHUNK // CPG              # 2 groups per full block
GJ = CPG * JCHUNK                   # 128 (c,q,j) groups per tree
BGRP = CPG * BQ * JBANK             # 96 ScalarE-path (c,q,j) groups per tree
GSPLIT = 72                         # tree level-1 groups handled by GpSimd
JK = K * B                          # 12800 diff columns per full block
# ba scratch: tree level regions for 96 groups (25+12+6+2+1 wide + singles)
BA_COLS = 4864
# bb scratch: ScalarE abs output, 96 groups x 50
BB_COLS = BGRP * K

_cache = {}


def _build_program():
    import concourse.mybir as mybir
    from concourse import bacc, tile
    from concourse.masks import make_identity

    f32 = mybir.dt.float32
    bf16 = mybir.dt.bfloat16
    fp8 = mybir.dt.float8e4
    Alu = mybir.AluOpType
    Act = mybir.ActivationFunctionType

    nc = bacc.Bacc("TRN2", target_bir_lowering=False, debug=False,
                   enable_asserts=False)

    # host layouts chosen for >=2KB DMA partition lines
    xT_d = nc.dram_tensor("xT", [P, CC * B], fp8, kind="ExternalInput").ap()
    T_d = nc.dram_tensor("Tl", [P, CC * N_LOC], fp8, kind="ExternalInput").ap()
    rp_d = nc.dram_tensor("rp", [K + 2, JK], fp8, kind="ExternalInput").ap()
    feat_d = nc.dram_tensor("feat", [B, O_LOC], f32, kind="ExternalOutput").ap()

    CH = QB * 512                   # 2048 PSUM elements per chunk

    with tile.TileContext(nc) as tc:
        with (
            tc.tile_pool(name="static", bufs=1) as static,
            tc.tile_pool(name="babsp", bufs=4) as babsp,
            tc.tile_pool(name="bbp", bufs=4) as bbp,
            tc.tile_pool(name="dexpp", bufs=3) as dexpp,
            tc.tile_pool(name="gatep", bufs=2) as gatep,
            tc.tile_pool(name="et0p", bufs=8) as et0p,
            tc.tile_pool(name="et1p", bufs=2) as et1p,
            tc.tile_pool(name="dramp", bufs=1, space="DRAM") as dramp,
        ):
            # ---- stage 1 inputs load first: they gate the M' matmul --------
            # single wide transfers: the 2-3KB partition lines run at full
            # DMA rate, whereas per-chunk slices (256-400B lines) measured
            # ~9 GB/s and pushed the first matmul out to ~10us
            engs = [nc.sync, nc.scalar]
            xt_sb = static.tile([P, CC * B], fp8, tag="xt")
            t_sb = static.tile([P, CC * N_LOC], fp8, tag="t")
            th = CC * N_LOC // 2
            nc.scalar.dma_start(out=t_sb[:, 0:th], in_=T_d[:, 0:th])
            nc.sync.dma_start(out=xt_sb[:, :], in_=xT_d[:, :])
            nc.sync.dma_start(out=t_sb[:, th:], in_=T_d[:, th:])

            # rhs I-parts follow; they are not needed until stage 4
            rhs_t = []
            for h in range(2):
                rt = static.tile([K + 1, JK], fp8, tag=f"rhs{h}",
                                 name=f"rhs{h}")
                qw = JK // 2
                for s in range(2):
                    engs[s].dma_start(
                        out=rt[0:K, s * qw:(s + 1) * qw],
                        in_=rp_d[0:K, s * qw:(s + 1) * qw])
                rhs_t.append(rt)

            warm = static.tile([1, 2], f32, tag="warm")
            nc.vector.memset(warm[:, :], 0.0)
            nc.scalar.activation(out=warm[:, :], in_=warm[:, :],
                                 func=Act.Exp, scale=-1.0)
            ident = static.tile([P, P], bf16, tag="ident")
            make_identity(nc, ident[:, :])
            identf = static.tile([JBANK, JBANK], f32, tag="identf")
            make_identity(nc, identf[:, :])
            ones_col = static.tile([P, 1], bf16, tag="ones_col")
            nc.vector.memset(ones_col[:, :], 1.0)

            # +M' staged to DRAM as one flat j-major row per o, so the
            # per-o rhs row refresh is a single contiguous 12.8KB packet
            posm_d = dramp.tile([O_LOC, JK], fp8, tag="posm_d")
            m_bf = []
            m_bb = []
            with tc.tile_pool(name="mmp", bufs=2, space="PSUM") as mmp:
                for it in range(ITILES):
                    pm = mmp.tile([P, N_LOC], f32, tag="pm")
                    for cc in range(CC):
                        nc.tensor.matmul(
                            pm[:, :],
                            lhsT=xt_sb[:, cc * B + it * P: cc * B + it * P + P],
                            rhs=t_sb[:, cc * N_LOC:(cc + 1) * N_LOC],
                            start=(cc == 0), stop=(cc == CC - 1),
                        )
                    mb = static.tile([P, N_LOC], fp8, tag=f"mbf{it}",
                                     name=f"mbf{it}")
                    nc.scalar.copy(mb[:, :], pm[:, :])
                    m_bf.append(mb)
                    mbb = static.tile([P, N_LOC], bf16, tag=f"mbb{it}",
                                      name=f"mbb{it}")
                    nc.scalar.copy(mbb[:, :], mb[:, :])
                    m_bb.append(mbb)
                half = K * P
                for o in range(O_LOC):
                    for it in range(ITILES):
                        engs[(o + it) % 2].dma_start(
                            out=posm_d[o:o + 1,
                                       it * half:(it + 1) * half],
                            in_=m_bf[it][:, o * K:(o + 1) * K])

            # ---- stage 2: lhsT tiles [M'_o^T (50 rows); -ones] -------------
            # the -1 row arrives by DMA from rp row 51 (partition 50 is
            # not engine-alignable)
            lhs = []
            with tc.tile_pool(name="tpp", bufs=2, space="PSUM") as tpp:
                for o in range(O_LOC):
                    lt = static.tile([K + 1, B], fp8, tag=f"lhs{o}",
                                     name=f"lhs{o}")
                    for it in range(ITILES):
                        tp = tpp.tile([K, P], bf16, tag="tp")
                        nc.tensor.transpose(
                            tp[:, :], m_bb[it][:, o * K: o * K + K],
                            ident[:, :])
                        nc.scalar.copy(lt[0:K, it * P:(it + 1) * P], tp[:, :])
                    nc.sync.dma_start(out=lt[K:K + 1, 0:B],
                                      in_=rp_d[K + 1:K + 2, 0:B])
                    lhs.append(lt)

            # ---- stage 4: per (o, itile): diffs -> max(|p|,|m|) -> tree ----
            feat_sb = [static.tile([P, O_LOC], f32, tag=f"feat{it}",
                                   name=f"feat{it}")
                       for it in range(ITILES)]
            et0_tiles = []
            stage4 = tc.tile_pool(name="chp", bufs=1, space="PSUM")
            chp = stage4.__enter__()
            # one 8-bank PSUM ring; subtile dependency tracking lets each
            # 4-bank half recycle as soon as ITS consumers finish, instead
            # of chunk n+2 waiting on chunk n's full-tile release
            ring = chp.tile([P, 2 * CH], f32, tag="ring")
            ci = 0
            def emit_tree(ba, bb, gsl):
                """Return thunks emitting the reduction tree for one group."""
                thunks = []
                gslc = gsl.rearrange("p (c r) -> p c r", c=CPG)

                def view(ofs, width):
                    return ba[:, ofs: ofs + BGRP * width].rearrange(
                        "p (g k) -> p g k", k=width)
                bbv = bb[:, :].rearrange("p (g k) -> p g k", k=K)
                thunks.append(lambda: nc.vector.tensor_tensor(
                    out=view(0, KH), in0=bbv[:, :, 0:KH],
                    in1=bbv[:, :, KH:K], op=Alu.add))
                cur, w = 0, KH
                free = BGRP * KH
                singles = []
                while w > 1:
                    hw = w // 2
                    src_, fr = view(cur, w), free
                    rem = w - 2 * hw
                    if rem == 1:
                        singles.append(src_[:, :, w - 1:w])
                    elif rem == 2:
                        thunks.append(
                            lambda s=src_, f=fr, ww=w: nc.vector.tensor_tensor(
                                out=view(f, 1), in0=s[:, :, ww - 2:ww - 1],
                                in1=s[:, :, ww - 1:ww], op=Alu.add))
                        singles.append(view(free, 1))
                        free += BGRP
                        fr = free
                    thunks.append(
                        lambda s=src_, f=fr, h=hw: nc.vector.tensor_tensor(
                            out=view(f, h), in0=s[:, :, 0:h],
                            in1=s[:, :, h:2 * h], op=Alu.add))
                    cur = fr
                    free = fr + hw * BGRP
                    w = hw
                # final merge writes the (c, q>=ABANK, j) gsl slots
                gslb = gslc[:, :, ABANK * JBANK:JCHUNK]

                def asb(v):
                    return v.rearrange("p (c r) k -> p c (r k)", c=CPG)
                for si in range(len(singles)):
                    last = si == len(singles) - 1
                    cu, fr, sv = cur, free, singles[si]
                    if last:
                        thunks.append(lambda c_=cu, s=sv: nc.vector.tensor_tensor(
                            out=gslb, in0=asb(view(c_, 1)), in1=asb(s),
                            op=Alu.add))
                    else:
                        thunks.append(
                            lambda c_=cu, f=fr, s=sv: nc.vector.tensor_tensor(
                                out=view(f, 1), in0=view(c_, 1), in1=s,
                                op=Alu.add))
                    cur = free
                    free += BGRP
                if not singles:
                    thunks.append(lambda c_=cur: nc.vector.tensor_copy(
                        out=gslb, in_=asb(view(c_, 1))))
                return thunks

            pending = []
            for o in range(O_LOC):
                rt = rhs_t[o % 2]
                nc.sync.dma_start(out=rt[K:K + 1, :],
                                  in_=posm_d[o:o + 1, :])
                for it in range(ITILES):
                    g_lo = 0 if it == 0 else NGROUP // 2
                    nj = (NGROUP - g_lo) * GJ
                    dexp = dexpp.tile([P, B], f32, tag="dexp")
                    for g in range(g_lo, NGROUP):
                        gsl = dexp[:, (g - g_lo) * GJ:(g - g_lo + 1) * GJ]
                        gslc = gsl.rearrange("p (c r) -> p c r", c=CPG)
                        ba = babsp.tile([P, BA_COLS], bf16, tag="ba")
                        bb = bbp.tile([P, BB_COLS], bf16, tag="bb")
                        for cc in range(CPG):
                            c = g * CPG + cc
                            ch = ring[:, (ci % 2) * CH:(ci % 2 + 1) * CH]
                            ci += 1
                            # B banks first: the wide ScalarE Abs can start
                            # a matmul earlier, shortening PSUM-free latency
                            for q in list(range(ABANK, QB)) + list(range(ABANK)):
                                col = (c * JCHUNK + q * JBANK) * K
                                nc.tensor.matmul(
                                    ch[:, q * 512: q * 512 + JBANK * K],
                                    lhsT=lhs[o][:, it * P:(it + 1) * P],
                                    rhs=rt[:, col: col + JBANK * K],
                                    start=True, stop=True)
                            # PSUM chunk viewed [p, q, j(8), k(50)]
                            ch4 = ch[:, :].rearrange(
                                "p (q r) -> p q r", q=QB)[
                                :, :, 0:JBANK * K].rearrange(
                                "p q (j k) -> p q j k", k=K)
                            # PSUM-freeing consumers run at high priority so
                            # the scheduler issues them ahead of queued tree
                            # adds (otherwise the PE stalls on PSUM recycle
                            # behind a tree lump in the DVE FIFO)
                            with tc.high_priority():
                                # A bank: DVE fused |.| + k-reduce from PSUM
                                nc.vector.tensor_reduce(
                                    out=gslc[:, cc:cc + 1, 0:ABANK * JBANK],
                                    in_=ch4[:, 0:ABANK, :, :],
                                    axis=mybir.AxisListType.X, op=Alu.add,
                                    apply_absolute_value=True)
                                # B banks: ScalarE |.| -> bf16 dense (j,k50)
                                nc.scalar.activation(
                                    out=bb[:, cc * BQ * JBANK * K:
                                           (cc + 1) * BQ * JBANK
                                           * K].rearrange(
                                        "p (q j k) -> p q j k",
                                        q=BQ, j=JBANK),
                                    in_=ch4[:, ABANK:QB, :, :], func=Act.Abs)
                            if pending and g == g_lo and cc == 0:
                                # release the previous block's deferred exp,
                                # gated on THIS chunk's Abs output so the
                                # scheduler cannot slot the exp+accum-read
                                # lump ahead of a PSUM-freeing Abs
                                pending.pop()(bb)
                        for t in emit_tree(ba, bb, gsl):
                            t()
                    if it == 0:
                        et = et0p.tile([P, B], bf16, tag="et0",
                                       name=f"et0_{o}")
                        et0_tiles.append(et)
                    else:
                        et = et1p.tile([P, B // 2], bf16, tag="et1")

                    # exp(-l1) + fused j-sum into the feature column,
                    # deferred into the next block's first chunk
                    def mk_exp(et=et, dexp=dexp, nj=nj, it=it, o=o):
                        def emit(gate_bb):
                            bias = 0.0
                            if gate_bb is not None:
                                gate = gatep.tile([P, 1], f32, tag="gate")
                                nc.vector.tensor_scalar(
                                    out=gate[:, :], in0=gate_bb[:, 0:1],
                                    scalar1=0.0, scalar2=None, op0=Alu.mult)
                                bias = gate[:, :]
                            nc.scalar.activation(
                                out=et[:, :], in_=dexp[:, 0:nj],
                                func=Act.Exp, scale=-1.0, bias=bias,
                                accum_out=feat_sb[it][:, o:o + 1])
                        return emit
                    pending.append(mk_exp())
            if pending:
                pending.pop()(None)
            stage4.__exit__(None, None, None)

            # ---- stage 5: mirrored contribution for itile 1 ----------------
            # colsum_o[j] = sum_{i in it0} exp(-D[i, j]) for j in [128, 256)
            cs_sb = static.tile([JBANK, P], f32, tag="cs_sb")
            with tc.tile_pool(name="csp", bufs=2, space="PSUM") as csp:
                for o in range(O_LOC):
                    cs = csp.tile([1, P], f32, tag="cs")
                    nc.tensor.matmul(cs[:, :], lhsT=ones_col[:, :],
                                     rhs=et0_tiles[o][:, P:B],
                                     start=True, stop=True)
                    cs_row = babsp.tile([1, P], f32, tag="cs_row")
                    nc.scalar.copy(cs_row[:, :], cs[:, :])
                    nc.sync.dma_start(out=cs_sb[o:o + 1, :], in_=cs_row[:, :])
                ct = csp.tile([P, JBANK], f32, tag="ct")
                nc.tensor.transpose(ct[:, :], cs_sb[:, :], identf[:, :])
                nc.vector.tensor_tensor(out=feat_sb[1][:, :],
                                        in0=feat_sb[1][:, :],
                                        in1=ct[:, :], op=Alu.add)

            for it in range(ITILES):
                nc.vector.tensor_scalar(
                    out=feat_sb[it][:, :], in0=feat_sb[it][:, :],
                    scalar1=1.0, scalar2=None, op0=Alu.subtract)
                nc.sync.dma_start(out=feat_d[it * P:(it + 1) * P, :],
                                  in_=feat_sb[it][:, :])

    nc.compile()
    return nc


def _get_program():
    if "nc" not in _cache:
        _cache["nc"] = _build_program()
    return _cache["nc"]


def prepare_in_maps(x, T):
    """Host-side sharding: transpose/cast x, pair-transform + slice T."""
    f8 = ml_dtypes.float8_e4m3fn
    xf = np.asarray(x, dtype=np.float32)
    # xT host layout [128, CC*B]: partition p holds x^T rows p+128*cc
    xT = np.ascontiguousarray(xf.T).reshape(CC, P, B).transpose(1, 0, 2)
    xT = np.ascontiguousarray(xT.reshape(P, CC * B)).astype(f8)
    Tp = np.asarray(T, dtype=np.float32).reshape(IN_FEATURES, O_TOTAL, K)
    in_maps = []
    rp = np.zeros((K + 2, JK), dtype=f8)
    kk = np.arange(K)
    for j in range(B):
        rp[kk, j * K + kk] = 1.0
    rp[K + 1, :] = -1.0
    for c in range(N_CORES):
        Tl = Tp[:, c * O_LOC:(c + 1) * O_LOC, :].reshape(IN_FEATURES, N_LOC)
        # host layout [128, CC*N_LOC]: partition p holds rows p+128*cc
        Tl = np.ascontiguousarray(Tl).reshape(CC, P, N_LOC).transpose(1, 0, 2)
        Tl = np.ascontiguousarray(Tl.reshape(P, CC * N_LOC)).astype(f8)
        in_maps.append({"xT": xT, "Tl": Tl, "rp": rp})
    return in_maps


def run_cores(in_maps, trace=False, tmpdir=None):
    from concourse import bass_utils
    nc = _get_program()
    return bass_utils.run_bass_kernel_spmd(
        nc, in_maps, core_ids=list(range(N_CORES)), trace=trace, tmpdir=tmpdir)


def kernel(x, T):
    x = np.asarray(x, dtype=np.float32)
    res = run_cores(prepare_in_maps(x, T))
    feat = np.concatenate(
        [res.results[c]["feat"].astype(np.float32) for c in range(N_CORES)],
        axis=1)
    return np.concatenate([x, feat], axis=1)


# revision 39
# speedup vs baseline: 1.0081x; 1.0081x over previous
"""Trainium2 Bass kernel for MinibatchDiscrimination.

Reference computation (B=256, IN=1024, O=64, K=50):
    M = (x @ T).reshape(B, O, K)
    l1[i,j,o] = sum_k |M[i,o,k] - M[j,o,k]|
    out = concat([x, sum_j exp(-l1) - 1], axis=1)          # [B, IN + O]

Sharding: the O (out_features) dimension is split across the 8 NeuronCores
(8 features per core); x is replicated. Each core computes its [256, 8]
feature block; the host gathers the blocks and concatenates with x.

Per-core pipeline:
  1. PE matmul: M[256, 400] = xT.T @ T_local (fp8 in, f32 PSUM), cast to
     fp8 -- the canonical value used on BOTH sides of the pairwise
     subtraction, so the diagonal distance is exactly zero.  +M is staged
     to DRAM as flat j-major rows.  Input DMAs are pipelined per
     contraction chunk so the matmul starts as soon as the first 1/8th
     of x and T arrive.
  2. All-pairs signed differences generated by the PE with an affine
     matmul: lhsT = [M_o^T (50 k-rows); -ones] and rhs = [I50 tiled over
     j; +M row].  Chunks of 32 j land in PSUM as [128, 4x512] f32.
  3. Symmetry: itile-1 blocks only compute j in [128,256); the mirrored
     contribution comes from PE column-sums of the itile-0 exp tiles.
  4. Consumers per 4-bank chunk, balanced across engines: bank 0 takes a
     fused DVE tensor_reduce(add, |.|) straight from PSUM into the l1
     slot; banks 1-3 take one wide ScalarE Abs -> bf16 SBUF, reduced
     50->1 by a dense DVE binary add-tree at bf16 2x rate (tree emitted
     immediately per 4-chunk group; interleaving or GpSimd offload both
     measured slower due to DVE FIFO ordering and SBUF-port contention).
  5. ScalarE exp(-l1) with accum_out producing the j-sum directly
     (no separate DVE reduction).
"""

import numpy as np
import ml_dtypes

B = 256
IN_FEATURES = 1024
O_TOTAL = 64
K = 50
KH = 25                             # k-pairs per feature
N_CORES = 8
O_LOC = O_TOTAL // N_CORES          # 8 features per core
N_LOC = O_LOC * K                   # 400 M' columns per core
P = 128                             # partitions
ITILES = B // P                     # 2 row tiles
CC = IN_FEATURES // P               # 8 contraction chunks
JCHUNK = 32                         # j's per PSUM chunk
JBANK = 8                           # j's per PSUM bank (8*50 = 400 of 512)
QB = JCHUNK // JBANK                # banks per chunk = 4
ABANK = 1                           # banks 0..ABANK-1 -> DVE direct-reduce
BQ = QB - ABANK                     # banks per chunk on the ScalarE path
NCHUNK = B // JCHUNK                # 8 chunks per full block
CPG = 4                             # chunks per tree group
NGROUP = NCHUNK // CPG              # 2 groups per full block
GJ = CPG * JCHUNK                   # 128 (c,q,j) groups per tree
BGRP = CPG * BQ * JBANK             # 96 ScalarE-path (c,q,j) groups per tree
GSPLIT = 72                         # tree level-1 groups handled by GpSimd
JK = K * B                          # 12800 diff columns per full block
# ba scratch: tree level regions for 96 groups (25+12+6+2+1 wide + singles)
BA_COLS = 4864
# bb scratch: ScalarE abs output, 96 groups x 50
BB_COLS = BGRP * K

_cache = {}


def _build_program():
    import concourse.mybir as mybir
    from concourse import bacc, tile
    from concourse.masks import make_identity

    f32 = mybir.dt.float32
    bf16 = mybir.dt.bfloat16
    fp8 = mybir.dt.float8e4
    Alu = mybir.AluOpType
    Act = mybir.ActivationFunctionType

    nc = bacc.Bacc("TRN2", target_bir_lowering=False, debug=False,
                   enable_asserts=False)

    # host layouts chosen for >=2KB DMA partition lines
    xT_d = nc.dram_tensor("xT", [P, CC * B], fp8, kind="ExternalInput").ap()
    T_d = nc.dram_tensor("Tl", [P, CC * N_LOC], fp8, kind="ExternalInput").ap()
    rp_d = nc.dram_tensor("rp", [K + 2, JK], fp8, kind="ExternalInput").ap()
    feat_d = nc.dram_tensor("feat", [B, O_LOC], f32, kind="ExternalOutput").ap()

    CH = QB * 512                   # 2048 PSUM elements per chunk

    with tile.TileContext(nc) as tc:
        with (
            tc.tile_pool(name="static", bufs=1) as static,
            tc.tile_pool(name="babsp", bufs=4) as babsp,
            tc.tile_pool(name="bbp", bufs=4) as bbp,
            tc.tile_pool(name="dexpp", bufs=3) as dexpp,
            tc.tile_pool(name="et0p", bufs=8) as et0p,
            tc.tile_pool(name="et1p", bufs=2) as et1p,
            tc.tile_pool(name="dramp", bufs=1, space="DRAM") as dramp,
        ):
            # ---- stage 1 inputs load first: they gate the M' matmul --------
            # single wide transfers: the 2-3KB partition lines run at full
            # DMA rate, whereas per-chunk slices (256-400B lines) measured
            # ~9 GB/s and pushed the first matmul out to ~10us
            engs = [nc.sync, nc.scalar]
            xt_sb = static.tile([P, CC * B], fp8, tag="xt")
            t_sb = static.tile([P, CC * N_LOC], fp8, tag="t")
            th = CC * N_LOC // 2
            nc.scalar.dma_start(out=t_sb[:, 0:th], in_=T_d[:, 0:th])
            nc.sync.dma_start(out=xt_sb[:, :], in_=xT_d[:, :])
            nc.sync.dma_start(out=t_sb[:, th:], in_=T_d[:, th:])

            # rhs I-parts follow; they are not needed until stage 4
            rhs_t = []
            for h in range(2):
                rt = static.tile([K + 1, JK], fp8, tag=f"rhs{h}",
                                 name=f"rhs{h}")
                qw = JK // 2
                for s in range(2):
                    engs[s].dma_start(
                        out=rt[0:K, s * qw:(s + 1) * qw],
                        in_=rp_d[0:K, s * qw:(s + 1) * qw])
                rhs_t.append(rt)

            warm = static.tile([1, 2], f32, tag="warm")
            nc.vector.memset(warm[:, :], 0.0)
            nc.scalar.activation(out=warm[:, :], in_=warm[:, :],
                                 func=Act.Exp, scale=-1.0)
            ident = static.tile([P, P], bf16, tag="ident")
            make_identity(nc, ident[:, :])
            identf = static.tile([JBANK, JBANK], f32, tag="identf")
            make_identity(nc, identf[:, :])
            ones_col = static.tile([P, 1], bf16, tag="ones_col")
            nc.vector.memset(ones_col[:, :], 1.0)

            # +M' staged to DRAM as one flat j-major row per o, so the
            # per-o rhs row refresh is a single contiguous 12.8KB packet
            posm_d = dramp.tile([O_LOC, JK], fp8, tag="posm_d")
            m_bf = []
            m_bb = []
            with tc.tile_pool(name="mmp", bufs=2, space="PSUM") as mmp:
                for it in range(ITILES):
                    pm = mmp.tile([P, N_LOC], f32, tag="pm")
                    for cc in range(CC):
                        nc.tensor.matmul(
                            pm[:, :],
                            lhsT=xt_sb[:, cc * B + it * P: cc * B + it * P + P],
                            rhs=t_sb[:, cc * N_LOC:(cc + 1) * N_LOC],
                            start=(cc == 0), stop=(cc == CC - 1),
                        )
                    mb = static.tile([P, N_LOC], fp8, tag=f"mbf{it}",
                                     name=f"mbf{it}")
                    nc.scalar.copy(mb[:, :], pm[:, :])
                    m_bf.append(mb)
                    mbb = static.tile([P, N_LOC], bf16, tag=f"mbb{it}",
                                      name=f"mbb{it}")
                    nc.scalar.copy(mbb[:, :], mb[:, :])
                    m_bb.append(mbb)
                half = K * P
                for o in range(O_LOC):
                    for it in range(ITILES):
                        engs[(o + it) % 2].dma_start(
                            out=posm_d[o:o + 1,
                                       it * half:(it + 1) * half],
                            in_=m_bf[it][:, o * K:(o + 1) * K])

            # ---- stage 2: lhsT tiles [M'_o^T (50 rows); -ones] -------------
            # the -1 row arrives by DMA from rp row 51 (partition 50 is
            # not engine-alignable)
            lhs = []
            with tc.tile_pool(name="tpp", bufs=2, space="PSUM") as tpp:
                for o in range(O_LOC):
                    lt = static.tile([K + 1, B], fp8, tag=f"lhs{o}",
                                     name=f"lhs{o}")
                    for it in range(ITILES):
                        tp = tpp.tile([K, P], bf16, tag="tp")
                        nc.tensor.transpose(
                            tp[:, :], m_bb[it][:, o * K: o * K + K],
                            ident[:, :])
                        nc.scalar.copy(lt[0:K, it * P:(it + 1) * P], tp[:, :])
                    nc.sync.dma_start(out=lt[K:K + 1, 0:B],
                                      in_=rp_d[K + 1:K + 2, 0:B])
                    lhs.append(lt)

            # ---- stage 4: per (o, itile): diffs -> max(|p|,|m|) -> tree ----
            feat_sb = [static.tile([P, O_LOC], f32, tag=f"feat{it}",
                                   name=f"feat{it}")
                       for it in range(ITILES)]
            et0_tiles = []
            stage4 = tc.tile_pool(name="chp", bufs=1, space="PSUM")
            chp = stage4.__enter__()
            # one 8-bank PSUM ring; subtile dependency tracking lets each
            # 4-bank half recycle as soon as ITS consumers finish, instead
            # of chunk n+2 waiting on chunk n's full-tile release
            ring = chp.tile([P, 2 * CH], f32, tag="ring")
            ci = 0
            def emit_tree(ba, bb, gsl):
                """Return thunks emitting the reduction tree for one group."""
                thunks = []
                gslc = gsl.rearrange("p (c r) -> p c r", c=CPG)

                def view(ofs, width):
                    return ba[:, ofs: ofs + BGRP * width].rearrange(
                        "p (g k) -> p g k", k=width)
                bbv = bb[:, :].rearrange("p (g k) -> p g k", k=K)
                thunks.append(lambda: nc.vector.tensor_tensor(
                    out=view(0, KH), in0=bbv[:, :, 0:KH],
                    in1=bbv[:, :, KH:K], op=Alu.add))
                cur, w = 0, KH
                free = BGRP * KH
                singles = []
                while w > 1:
                    hw = w // 2
                    src_, fr = view(cur, w), free
                    rem = w - 2 * hw
                    if rem == 1:
                        singles.append(src_[:, :, w - 1:w])
                    elif rem == 2:
                        thunks.append(
                            lambda s=src_, f=fr, ww=w: nc.vector.tensor_tensor(
                                out=view(f, 1), in0=s[:, :, ww - 2:ww - 1],
                                in1=s[:, :, ww - 1:ww], op=Alu.add))
                        singles.append(view(free, 1))
                        free += BGRP
                        fr = free
                    thunks.append(
                        lambda s=src_, f=fr, h=hw: nc.vector.tensor_tensor(
                            out=view(f, h), in0=s[:, :, 0:h],
                            in1=s[:, :, h:2 * h], op=Alu.add))
                    cur = fr
                    free = fr + hw * BGRP
                    w = hw
                # final merge writes the (c, q>=ABANK, j) gsl slots
                gslb = gslc[:, :, ABANK * JBANK:JCHUNK]

                def asb(v):
                    return v.rearrange("p (c r) k -> p c (r k)", c=CPG)
                for si in range(len(singles)):
                    last = si == len(singles) - 1
                    cu, fr, sv = cur, free, singles[si]
                    if last:
                        thunks.append(lambda c_=cu, s=sv: nc.vector.tensor_tensor(
                            out=gslb, in0=asb(view(c_, 1)), in1=asb(s),
                            op=Alu.add))
                    else:
                        thunks.append(
                            lambda c_=cu, f=fr, s=sv: nc.vector.tensor_tensor(
                                out=view(f, 1), in0=view(c_, 1), in1=s,
                                op=Alu.add))
                    cur = free
                    free += BGRP
                if not singles:
                    thunks.append(lambda c_=cur: nc.vector.tensor_copy(
                        out=gslb, in_=asb(view(c_, 1))))
                return thunks

            pending = []
            for o in range(O_LOC):
                rt = rhs_t[o % 2]
                nc.sync.dma_start(out=rt[K:K + 1, :],
                                  in_=posm_d[o:o + 1, :])
                for it in range(ITILES):
                    g_lo = 0 if it == 0 else NGROUP // 2
                    nj = (NGROUP - g_lo) * GJ
                    dexp = dexpp.tile([P, B], f32, tag="dexp")
                    for g in range(g_lo, NGROUP):
                        gsl = dexp[:, (g - g_lo) * GJ:(g - g_lo + 1) * GJ]
                        gslc = gsl.rearrange("p (c r) -> p c r", c=CPG)
                        ba = babsp.tile([P, BA_COLS], bf16, tag="ba")
                        bb = bbp.tile([P, BB_COLS], bf16, tag="bb")
                        for cc in range(CPG):
                            c = g * CPG + cc
                            ch = ring[:, (ci % 2) * CH:(ci % 2 + 1) * CH]
                            ci += 1
                            # B banks first: the wide ScalarE Abs can start
                            # a matmul earlier, shortening PSUM-free latency
                            for q in list(range(ABANK, QB)) + list(range(ABANK)):
                                col = (c * JCHUNK + q * JBANK) * K
                                nc.tensor.matmul(
                                    ch[:, q * 512: q * 512 + JBANK * K],
                                    lhsT=lhs[o][:, it * P:(it + 1) * P],
                                    rhs=rt[:, col: col + JBANK * K],
                                    start=True, stop=True)
                            # PSUM chunk viewed [p, q, j(8), k(50)]
                            ch4 = ch[:, :].rearrange(
                                "p (q r) -> p q r", q=QB)[
                                :, :, 0:JBANK * K].rearrange(
                                "p q (j k) -> p q j k", k=K)
                            # PSUM-freeing consumers run at high priority so
                            # the scheduler issues them ahead of queued tree
                            # adds (otherwise the PE stalls on PSUM recycle
                            # behind a tree lump in the DVE FIFO)
                            with tc.high_priority():
                                # A bank: DVE fused |.| + k-reduce from PSUM
                                nc.vector.tensor_reduce(
                                    out=gslc[:, cc:cc + 1, 0:ABANK * JBANK],
                                    in_=ch4[:, 0:ABANK, :, :],
                                    axis=mybir.AxisListType.X, op=Alu.add,
                                    apply_absolute_value=True)
                                # B banks: ScalarE |.| -> bf16 dense (j,k50)
                                nc.scalar.activation(
                                    out=bb[:, cc * BQ * JBANK * K:
                                           (cc + 1) * BQ * JBANK
                                           * K].rearrange(
                                        "p (q j k) -> p q j k",
                                        q=BQ, j=JBANK),
                                    in_=ch4[:, ABANK:QB, :, :], func=Act.Abs)
                        for t in emit_tree(ba, bb, gsl):
                            t()
                    if it == 0:
                        et = et0p.tile([P, B], bf16, tag="et0",
                                       name=f"et0_{o}")
                        et0_tiles.append(et)
                    else:
                        et = et1p.tile([P, B // 2], bf16, tag="et1")
                    # exp(-l1) + fused j-sum into the feature column
                    nc.scalar.activation(out=et[:, :], in_=dexp[:, 0:nj],
                                         func=Act.Exp, scale=-1.0,
                                         accum_out=feat_sb[it][:, o:o + 1])
            stage4.__exit__(None, None, None)

            # ---- stage 5: mirrored contribution for itile 1 ----------------
            # colsum_o[j] = sum_{i in it0} exp(-D[i, j]) for j in [128, 256)
            cs_sb = static.tile([JBANK, P], f32, tag="cs_sb")
            with tc.tile_pool(name="csp", bufs=2, space="PSUM") as csp:
                for o in range(O_LOC):
                    cs = csp.tile([1, P], f32, tag="cs")
                    nc.tensor.matmul(cs[:, :], lhsT=ones_col[:, :],
                                     rhs=et0_tiles[o][:, P:B],
                                     start=True, stop=True)
                    cs_row = babsp.tile([1, P], f32, tag="cs_row")
                    nc.scalar.copy(cs_row[:, :], cs[:, :])
                    nc.sync.dma_start(out=cs_sb[o:o + 1, :], in_=cs_row[:, :])
                ct = csp.tile([P, JBANK], f32, tag="ct")
                nc.tensor.transpose(ct[:, :], cs_sb[:, :], identf[:, :])
                nc.vector.tensor_tensor(out=feat_sb[1][:, :],
                                        in0=feat_sb[1][:, :],
                                        in1=ct[:, :], op=Alu.add)

            for it in range(ITILES):
                nc.vector.tensor_scalar(
                    out=feat_sb[it][:, :], in0=feat_sb[it][:, :],
                    scalar1=1.0, scalar2=None, op0=Alu.subtract)
                nc.sync.dma_start(out=feat_d[it * P:(it + 1) * P, :],
                                  in_=feat_sb[it][:, :])

    nc.compile()
    return nc


def _get_program():
    if "nc" not in _cache:
        _cache["nc"] = _build_program()
    return _cache["nc"]


def prepare_in_maps(x, T):
    """Host-side sharding: transpose/cast x, pair-transform + slice T."""
    f8 = ml_dtypes.float8_e4m3fn
    xf = np.asarray(x, dtype=np.float32)
    # xT host layout [128, CC*B]: partition p holds x^T rows p+128*cc
    xT = np.ascontiguousarray(xf.T).reshape(CC, P, B).transpose(1, 0, 2)
    xT = np.ascontiguousarray(xT.reshape(P, CC * B)).astype(f8)
    Tp = np.asarray(T, dtype=np.float32).reshape(IN_FEATURES, O_TOTAL, K)
    in_maps = []
    rp = np.zeros((K + 2, JK), dtype=f8)
    kk = np.arange(K)
    for j in range(B):
        rp[kk, j * K + kk] = 1.0
    rp[K + 1, :] = -1.0
    for c in range(N_CORES):
        Tl = Tp[:, c * O_LOC:(c + 1) * O_LOC, :].reshape(IN_FEATURES, N_LOC)
        # host layout [128, CC*N_LOC]: partition p holds rows p+128*cc
        Tl = np.ascontiguousarray(Tl).reshape(CC, P, N_LOC).transpose(1, 0, 2)
        Tl = np.ascontiguousarray(Tl.reshape(P, CC * N_LOC)).astype(f8)
        in_maps.append({"xT": xT, "Tl": Tl, "rp": rp})
    return in_maps


def run_cores(in_maps, trace=False, tmpdir=None):
    from concourse import bass_utils
    nc = _get_program()
    return bass_utils.run_bass_kernel_spmd(
        nc, in_maps, core_ids=list(range(N_CORES)), trace=trace, tmpdir=tmpdir)


def kernel(x, T):
    x = np.asarray(x, dtype=np.float32)
    res = run_cores(prepare_in_maps(x, T))
    feat = np.concatenate(
        [res.results[c]["feat"].astype(np.float32) for c in range(N_CORES)],
        axis=1)
    return np.concatenate([x, feat], axis=1)


# revision 41
# speedup vs baseline: 1.0175x; 1.0093x over previous
"""Trainium2 Bass kernel for MinibatchDiscrimination.

Reference computation (B=256, IN=1024, O=64, K=50):
    M = (x @ T).reshape(B, O, K)
    l1[i,j,o] = sum_k |M[i,o,k] - M[j,o,k]|
    out = concat([x, sum_j exp(-l1) - 1], axis=1)          # [B, IN + O]

Sharding: the O (out_features) dimension is split across the 8 NeuronCores
(8 features per core); x is replicated. Each core computes its [256, 8]
feature block; the host gathers the blocks and concatenates with x.

Per-core pipeline:
  1. PE matmul: M[256, 400] = xT.T @ T_local (fp8 in, f32 PSUM), cast to
     fp8 -- the canonical value used on BOTH sides of the pairwise
     subtraction, so the diagonal distance is exactly zero.  +M is staged
     to DRAM as flat j-major rows.  Input DMAs are pipelined per
     contraction chunk so the matmul starts as soon as the first 1/8th
     of x and T arrive.
  2. All-pairs signed differences generated by the PE with an affine
     matmul: lhsT = [M_o^T (50 k-rows); -ones] and rhs = [I50 tiled over
     j; +M row].  Chunks of 32 j land in PSUM as [128, 4x512] f32.
  3. Symmetry: itile-1 blocks only compute j in [128,256); the mirrored
     contribution comes from PE column-sums of the itile-0 exp tiles.
  4. Consumers per 4-bank chunk, balanced across engines: bank 0 takes a
     fused DVE tensor_reduce(add, |.|) straight from PSUM into the l1
     slot; banks 1-3 take one wide ScalarE Abs -> bf16 SBUF, reduced
     50->1 by a dense DVE binary add-tree at bf16 2x rate (tree emitted
     immediately per 4-chunk group; interleaving or GpSimd offload both
     measured slower due to DVE FIFO ordering and SBUF-port contention).
  5. ScalarE exp(-l1) with accum_out producing the j-sum directly
     (no separate DVE reduction).
"""

import numpy as np
import ml_dtypes

B = 256
IN_FEATURES = 1024
O_TOTAL = 64
K = 50
KH = 25                             # k-pairs per feature
N_CORES = 8
O_LOC = O_TOTAL // N_CORES          # 8 features per core
N_LOC = O_LOC * K                   # 400 M' columns per core
P = 128                             # partitions
ITILES = B // P                     # 2 row tiles
CC = IN_FEATURES // P               # 8 contraction chunks
JCHUNK = 32                         # j's per PSUM chunk
JBANK = 8                           # j's per PSUM bank (8*50 = 400 of 512)
QB = JCHUNK // JBANK                # banks per chunk = 4
ABANK = 2                           # banks 0..ABANK-1 -> DVE direct-reduce
BQ = QB - ABANK                     # banks per chunk on the ScalarE path
NCHUNK = B // JCHUNK                # 8 chunks per full block
CPG = 4                             # chunks per tree group
NGROUP = NCHUNK // CPG              # 2 groups per full block
GJ = CPG * JCHUNK                   # 128 (c,q,j) groups per tree
BGRP = CPG * BQ * JBANK             # 96 ScalarE-path (c,q,j) groups per tree
GSPLIT = 72                         # tree level-1 groups handled by GpSimd
JK = K * B                          # 12800 diff columns per full block
# ba scratch: tree level regions for 96 groups (25+12+6+2+1 wide + singles)
BA_COLS = 4864
# bb scratch: ScalarE abs output, 96 groups x 50
BB_COLS = BGRP * K

_cache = {}


def _build_program():
    import concourse.mybir as mybir
    from concourse import bacc, tile
    from concourse.masks import make_identity

    f32 = mybir.dt.float32
    bf16 = mybir.dt.bfloat16
    fp8 = mybir.dt.float8e4
    Alu = mybir.AluOpType
    Act = mybir.ActivationFunctionType

    nc = bacc.Bacc("TRN2", target_bir_lowering=False, debug=False,
                   enable_asserts=False)

    # host layouts chosen for >=2KB DMA partition lines
    xT_d = nc.dram_tensor("xT", [P, CC * B], fp8, kind="ExternalInput").ap()
    T_d = nc.dram_tensor("Tl", [P, CC * N_LOC], fp8, kind="ExternalInput").ap()
    rp_d = nc.dram_tensor("rp", [K + 2, JK], fp8, kind="ExternalInput").ap()
    feat_d = nc.dram_tensor("feat", [B, O_LOC], f32, kind="ExternalOutput").ap()

    CH = QB * 512                   # 2048 PSUM elements per chunk

    with tile.TileContext(nc) as tc:
        with (
            tc.tile_pool(name="static", bufs=1) as static,
            tc.tile_pool(name="babsp", bufs=4) as babsp,
            tc.tile_pool(name="bbp", bufs=4) as bbp,
            tc.tile_pool(name="dexpp", bufs=3) as dexpp,
            tc.tile_pool(name="et0p", bufs=8) as et0p,
            tc.tile_pool(name="et1p", bufs=2) as et1p,
            tc.tile_pool(name="dramp", bufs=1, space="DRAM") as dramp,
        ):
            # ---- stage 1 inputs load first: they gate the M' matmul --------
            # single wide transfers: the 2-3KB partition lines run at full
            # DMA rate, whereas per-chunk slices (256-400B lines) measured
            # ~9 GB/s and pushed the first matmul out to ~10us
            engs = [nc.sync, nc.scalar]
            xt_sb = static.tile([P, CC * B], fp8, tag="xt")
            t_sb = static.tile([P, CC * N_LOC], fp8, tag="t")
            th = CC * N_LOC // 2
            nc.scalar.dma_start(out=t_sb[:, 0:th], in_=T_d[:, 0:th])
            nc.sync.dma_start(out=xt_sb[:, :], in_=xT_d[:, :])
            nc.sync.dma_start(out=t_sb[:, th:], in_=T_d[:, th:])

            # rhs I-parts follow; they are not needed until stage 4
            rhs_t = []
            for h in range(2):
                rt = static.tile([K + 1, JK], fp8, tag=f"rhs{h}",
                                 name=f"rhs{h}")
                qw = JK // 2
                for s in range(2):
                    engs[s].dma_start(
                        out=rt[0:K, s * qw:(s + 1) * qw],
                        in_=rp_d[0:K, s * qw:(s + 1) * qw])
                rhs_t.append(rt)

            warm = static.tile([1, 2], f32, tag="warm")
            nc.vector.memset(warm[:, :], 0.0)
            nc.scalar.activation(out=warm[:, :], in_=warm[:, :],
                                 func=Act.Exp, scale=-1.0)
            ident = static.tile([P, P], bf16, tag="ident")
            make_identity(nc, ident[:, :])
            identf = static.tile([JBANK, JBANK], f32, tag="identf")
            make_identity(nc, identf[:, :])
            ones_col = static.tile([P, 1], bf16, tag="ones_col")
            nc.vector.memset(ones_col[:, :], 1.0)

            # +M' staged to DRAM as one flat j-major row per o, so the
            # per-o rhs row refresh is a single contiguous 12.8KB packet
            posm_d = dramp.tile([O_LOC, JK], fp8, tag="posm_d")
            m_bf = []
            m_bb = []
            with tc.tile_pool(name="mmp", bufs=2, space="PSUM") as mmp:
                for it in range(ITILES):
                    pm = mmp.tile([P, N_LOC], f32, tag="pm")
                    for cc in range(CC):
                        nc.tensor.matmul(
                            pm[:, :],
                            lhsT=xt_sb[:, cc * B + it * P: cc * B + it * P + P],
                            rhs=t_sb[:, cc * N_LOC:(cc + 1) * N_LOC],
                            start=(cc == 0), stop=(cc == CC - 1),
                        )
                    mb = static.tile([P, N_LOC], fp8, tag=f"mbf{it}",
                                     name=f"mbf{it}")
                    nc.scalar.copy(mb[:, :], pm[:, :])
                    m_bf.append(mb)
                    mbb = static.tile([P, N_LOC], bf16, tag=f"mbb{it}",
                                      name=f"mbb{it}")
                    nc.scalar.copy(mbb[:, :], mb[:, :])
                    m_bb.append(mbb)
                half = K * P
                for o in range(O_LOC):
                    for it in range(ITILES):
                        engs[(o + it) % 2].dma_start(
                            out=posm_d[o:o + 1,
                                       it * half:(it + 1) * half],
                            in_=m_bf[it][:, o * K:(o + 1) * K])

            # ---- stage 2: lhsT tiles [M'_o^T (50 rows); -ones] -------------
            # the -1 row arrives by DMA from rp row 51 (partition 50 is
            # not engine-alignable)
            lhs = []
            with tc.tile_pool(name="tpp", bufs=2, space="PSUM") as tpp:
                for o in range(O_LOC):
                    lt = static.tile([K + 1, B], fp8, tag=f"lhs{o}",
                                     name=f"lhs{o}")
                    for it in range(ITILES):
                        tp = tpp.tile([K, P], bf16, tag="tp")
                        nc.tensor.transpose(
                            tp[:, :], m_bb[it][:, o * K: o * K + K],
                            ident[:, :])
                        nc.scalar.copy(lt[0:K, it * P:(it + 1) * P], tp[:, :])
                    nc.sync.dma_start(out=lt[K:K + 1, 0:B],
                                      in_=rp_d[K + 1:K + 2, 0:B])
                    lhs.append(lt)

            # ---- stage 4: per (o, itile): diffs -> max(|p|,|m|) -> tree ----
            feat_sb = [static.tile([P, O_LOC], f32, tag=f"feat{it}",
                                   name=f"feat{it}")
                       for it in range(ITILES)]
            et0_tiles = []
            stage4 = tc.tile_pool(name="chp", bufs=1, space="PSUM")
            chp = stage4.__enter__()
            # one 8-bank PSUM ring; subtile dependency tracking lets each
            # 4-bank half recycle as soon as ITS consumers finish, instead
            # of chunk n+2 waiting on chunk n's full-tile release
            ring = chp.tile([P, 2 * CH], f32, tag="ring")
            ci = 0
            def emit_tree(ba, bb, gsl):
                """Return thunks emitting the reduction tree for one group."""
                thunks = []
                gslc = gsl.rearrange("p (c r) -> p c r", c=CPG)

                def view(ofs, width):
                    return ba[:, ofs: ofs + BGRP * width].rearrange(
                        "p (g k) -> p g k", k=width)
                bbv = bb[:, :].rearrange("p (g k) -> p g k", k=K)
                thunks.append(lambda: nc.vector.tensor_tensor(
                    out=view(0, KH), in0=bbv[:, :, 0:KH],
                    in1=bbv[:, :, KH:K], op=Alu.add))
                cur, w = 0, KH
                free = BGRP * KH
                singles = []
                while w > 1:
                    hw = w // 2
                    src_, fr = view(cur, w), free
                    rem = w - 2 * hw
                    if rem == 1:
                        singles.append(src_[:, :, w - 1:w])
                    elif rem == 2:
                        thunks.append(
                            lambda s=src_, f=fr, ww=w: nc.vector.tensor_tensor(
                                out=view(f, 1), in0=s[:, :, ww - 2:ww - 1],
                                in1=s[:, :, ww - 1:ww], op=Alu.add))
                        singles.append(view(free, 1))
                        free += BGRP
                        fr = free
                    thunks.append(
                        lambda s=src_, f=fr, h=hw: nc.vector.tensor_tensor(
                            out=view(f, h), in0=s[:, :, 0:h],
                            in1=s[:, :, h:2 * h], op=Alu.add))
                    cur = fr
                    free = fr + hw * BGRP
                    w = hw
                # final merge writes the (c, q>=ABANK, j) gsl slots
                gslb = gslc[:, :, ABANK * JBANK:JCHUNK]

                def asb(v):
                    return v.rearrange("p (c r) k -> p c (r k)", c=CPG)
                for si in range(len(singles)):
                    last = si == len(singles) - 1
                    cu, fr, sv = cur, free, singles[si]
                    if last:
                        thunks.append(lambda c_=cu, s=sv: nc.vector.tensor_tensor(
                            out=gslb, in0=asb(view(c_, 1)), in1=asb(s),
                            op=Alu.add))
                    else:
                        thunks.append(
                            lambda c_=cu, f=fr, s=sv: nc.vector.tensor_tensor(
                                out=view(f, 1), in0=view(c_, 1), in1=s,
                                op=Alu.add))
                    cur = free
                    free += BGRP
                if not singles:
                    thunks.append(lambda c_=cur: nc.vector.tensor_copy(
                        out=gslb, in_=asb(view(c_, 1))))
                return thunks

            pending = []
            for o in range(O_LOC):
                rt = rhs_t[o % 2]
                nc.sync.dma_start(out=rt[K:K + 1, :],
                                  in_=posm_d[o:o + 1, :])
                for it in range(ITILES):
                    g_lo = 0 if it == 0 else NGROUP // 2
                    nj = (NGROUP - g_lo) * GJ
                    dexp = dexpp.tile([P, B], f32, tag="dexp")
                    for g in range(g_lo, NGROUP):
                        gsl = dexp[:, (g - g_lo) * GJ:(g - g_lo + 1) * GJ]
                        gslc = gsl.rearrange("p (c r) -> p c r", c=CPG)
                        ba = babsp.tile([P, BA_COLS], bf16, tag="ba")
                        bb = bbp.tile([P, BB_COLS], bf16, tag="bb")
                        for cc in range(CPG):
                            c = g * CPG + cc
                            ch = ring[:, (ci % 2) * CH:(ci % 2 + 1) * CH]
                            ci += 1
                            # B banks first: the wide ScalarE Abs can start
                            # a matmul earlier, shortening PSUM-free latency
                            for q in list(range(ABANK, QB)) + list(range(ABANK)):
                                col = (c * JCHUNK + q * JBANK) * K
                                nc.tensor.matmul(
                                    ch[:, q * 512: q * 512 + JBANK * K],
                                    lhsT=lhs[o][:, it * P:(it + 1) * P],
                                    rhs=rt[:, col: col + JBANK * K],
                                    start=True, stop=True)
                            # PSUM chunk viewed [p, q, j(8), k(50)]
                            ch4 = ch[:, :].rearrange(
                                "p (q r) -> p q r", q=QB)[
                                :, :, 0:JBANK * K].rearrange(
                                "p q (j k) -> p q j k", k=K)
                            # PSUM-freeing consumers run at high priority so
                            # the scheduler issues them ahead of queued tree
                            # adds (otherwise the PE stalls on PSUM recycle
                            # behind a tree lump in the DVE FIFO)
                            with tc.high_priority():
                                # A banks: one DVE fused |.| + k-reduce
                                # from PSUM covering all direct banks
                                nc.vector.tensor_reduce(
                                    out=gsl[:, cc * JCHUNK:
                                            cc * JCHUNK
                                            + ABANK * JBANK].rearrange(
                                        "p (q j) -> p q j", q=ABANK),
                                    in_=ch4[:, 0:ABANK, :, :],
                                    axis=mybir.AxisListType.X, op=Alu.add,
                                    apply_absolute_value=True)
                                # B banks: ScalarE |.| -> bf16 dense (j,k50)
                                nc.scalar.activation(
                                    out=bb[:, cc * BQ * JBANK * K:
                                           (cc + 1) * BQ * JBANK
                                           * K].rearrange(
                                        "p (q j k) -> p q j k",
                                        q=BQ, j=JBANK),
                                    in_=ch4[:, ABANK:QB, :, :], func=Act.Abs)
                        for t in emit_tree(ba, bb, gsl):
                            t()
                    if it == 0:
                        et = et0p.tile([P, B], bf16, tag="et0",
                                       name=f"et0_{o}")
                        et0_tiles.append(et)
                    else:
                        et = et1p.tile([P, B // 2], bf16, tag="et1")
                    # exp(-l1) + fused j-sum into the feature column
                    nc.scalar.activation(out=et[:, :], in_=dexp[:, 0:nj],
                                         func=Act.Exp, scale=-1.0,
                                         accum_out=feat_sb[it][:, o:o + 1])
            stage4.__exit__(None, None, None)

            # ---- stage 5: mirrored contribution for itile 1 ----------------
            # colsum_o[j] = sum_{i in it0} exp(-D[i, j]) for j in [128, 256)
            cs_sb = static.tile([JBANK, P], f32, tag="cs_sb")
            with tc.tile_pool(name="csp", bufs=2, space="PSUM") as csp:
                for o in range(O_LOC):
                    cs = csp.tile([1, P], f32, tag="cs")
                    nc.tensor.matmul(cs[:, :], lhsT=ones_col[:, :],
                                     rhs=et0_tiles[o][:, P:B],
                                     start=True, stop=True)
                    cs_row = babsp.tile([1, P], f32, tag="cs_row")
                    nc.scalar.copy(cs_row[:, :], cs[:, :])
                    nc.sync.dma_start(out=cs_sb[o:o + 1, :], in_=cs_row[:, :])
                ct = csp.tile([P, JBANK], f32, tag="ct")
                nc.tensor.transpose(ct[:, :], cs_sb[:, :], identf[:, :])
                nc.vector.tensor_tensor(out=feat_sb[1][:, :],
                                        in0=feat_sb[1][:, :],
                                        in1=ct[:, :], op=Alu.add)

            for it in range(ITILES):
                nc.vector.tensor_scalar(
                    out=feat_sb[it][:, :], in0=feat_sb[it][:, :],
                    scalar1=1.0, scalar2=None, op0=Alu.subtract)
                nc.sync.dma_start(out=feat_d[it * P:(it + 1) * P, :],
                                  in_=feat_sb[it][:, :])

    nc.compile()
    return nc


def _get_program():
    if "nc" not in _cache:
        _cache["nc"] = _build_program()
    return _cache["nc"]


def prepare_in_maps(x, T):
    """Host-side sharding: transpose/cast x, pair-transform + slice T."""
    f8 = ml_dtypes.float8_e4m3fn
    xf = np.asarray(x, dtype=np.float32)
    # xT host layout [128, CC*B]: partition p holds x^T rows p+128*cc
    xT = np.ascontiguousarray(xf.T).reshape(CC, P, B).transpose(1, 0, 2)
    xT = np.ascontiguousarray(xT.reshape(P, CC * B)).astype(f8)
    Tp = np.asarray(T, dtype=np.float32).reshape(IN_FEATURES, O_TOTAL, K)
    in_maps = []
    rp = np.zeros((K + 2, JK), dtype=f8)
    kk = np.arange(K)
    for j in range(B):
        rp[kk, j * K + kk] = 1.0
    rp[K + 1, :] = -1.0
    for c in range(N_CORES):
        Tl = Tp[:, c * O_LOC:(c + 1) * O_LOC, :].reshape(IN_FEATURES, N_LOC)
        # host layout [128, CC*N_LOC]: partition p holds rows p+128*cc
        Tl = np.ascontiguousarray(Tl).reshape(CC, P, N_LOC).transpose(1, 0, 2)
        Tl = np.ascontiguousarray(Tl.reshape(P, CC * N_LOC)).astype(f8)
        in_maps.append({"xT": xT, "Tl": Tl, "rp": rp})
    return in_maps


def run_cores(in_maps, trace=False, tmpdir=None):
    from concourse import bass_utils
    nc = _get_program()
    return bass_utils.run_bass_kernel_spmd(
        nc, in_maps, core_ids=list(range(N_CORES)), trace=trace, tmpdir=tmpdir)


def kernel(x, T):
    x = np.asarray(x, dtype=np.float32)
    res = run_cores(prepare_in_maps(x, T))
    feat = np.concatenate(
        [res.results[c]["feat"].astype(np.float32) for c in range(N_CORES)],
        axis=1)
    return np.concatenate([x, feat], axis=1)


# revision 42
# speedup vs baseline: 1.0327x; 1.0150x over previous
"""Trainium2 Bass kernel for MinibatchDiscrimination.

Reference computation (B=256, IN=1024, O=64, K=50):
    M = (x @ T).reshape(B, O, K)
    l1[i,j,o] = sum_k |M[i,o,k] - M[j,o,k]|
    out = concat([x, sum_j exp(-l1) - 1], axis=1)          # [B, IN + O]

Sharding: the O (out_features) dimension is split across the 8 NeuronCores
(8 features per core); x is replicated. Each core computes its [256, 8]
feature block; the host gathers the blocks and concatenates with x.

Per-core pipeline:
  1. PE matmul: M[256, 400] = xT.T @ T_local (fp8 in, f32 PSUM), cast to
     fp8 -- the canonical value used on BOTH sides of the pairwise
     subtraction, so the diagonal distance is exactly zero.  +M is staged
     to DRAM as flat j-major rows.  Input DMAs are pipelined per
     contraction chunk so the matmul starts as soon as the first 1/8th
     of x and T arrive.
  2. All-pairs signed differences generated by the PE with an affine
     matmul: lhsT = [M_o^T (50 k-rows); -ones] and rhs = [I50 tiled over
     j; +M row].  Chunks of 32 j land in PSUM as [128, 4x512] f32.
  3. Symmetry: itile-1 blocks only compute j in [128,256); the mirrored
     contribution comes from PE column-sums of the itile-0 exp tiles.
  4. Consumers per 4-bank chunk, balanced across engines: bank 0 takes a
     fused DVE tensor_reduce(add, |.|) straight from PSUM into the l1
     slot; banks 1-3 take one wide ScalarE Abs -> bf16 SBUF, reduced
     50->1 by a dense DVE binary add-tree at bf16 2x rate (tree emitted
     immediately per 4-chunk group; interleaving or GpSimd offload both
     measured slower due to DVE FIFO ordering and SBUF-port contention).
  5. ScalarE exp(-l1) with accum_out producing the j-sum directly
     (no separate DVE reduction).
"""

import numpy as np
import ml_dtypes

B = 256
IN_FEATURES = 1024
O_TOTAL = 64
K = 50
KH = 25                             # k-pairs per feature
N_CORES = 8
O_LOC = O_TOTAL // N_CORES          # 8 features per core
N_LOC = O_LOC * K                   # 400 M' columns per core
P = 128                             # partitions
ITILES = B // P                     # 2 row tiles
CC = IN_FEATURES // P               # 8 contraction chunks
JCHUNK = 32                         # j's per PSUM chunk
JBANK = 8                           # j's per PSUM bank (8*50 = 400 of 512)
QB = JCHUNK // JBANK                # banks per chunk = 4
ABANK = 2                           # banks 0..ABANK-1 -> DVE direct-reduce
BQ = QB - ABANK                     # banks per chunk on the ScalarE path
NCHUNK = B // JCHUNK                # 8 chunks per full block
CPG = 4                             # chunks per tree group
NGROUP = NCHUNK // CPG              # 2 groups per full block
GJ = CPG * JCHUNK                   # 128 (c,q,j) groups per tree
JK = K * B                          # 12800 diff columns per full block
# ba scratch: tree level regions for up to 128 groups
BA_COLS = 6656
# bb scratch: ScalarE abs output, up to 128 groups x 50
BB_COLS = NCHUNK * BQ * JBANK * K

_cache = {}


def _build_program():
    import concourse.mybir as mybir
    from concourse import bacc, tile
    from concourse.masks import make_identity

    f32 = mybir.dt.float32
    bf16 = mybir.dt.bfloat16
    fp8 = mybir.dt.float8e4
    Alu = mybir.AluOpType
    Act = mybir.ActivationFunctionType

    nc = bacc.Bacc("TRN2", target_bir_lowering=False, debug=False,
                   enable_asserts=False)

    # host layouts chosen for >=2KB DMA partition lines
    xT_d = nc.dram_tensor("xT", [P, CC * B], fp8, kind="ExternalInput").ap()
    T_d = nc.dram_tensor("Tl", [P, CC * N_LOC], fp8, kind="ExternalInput").ap()
    rp_d = nc.dram_tensor("rp", [K + 2, JK], fp8, kind="ExternalInput").ap()
    feat_d = nc.dram_tensor("feat", [B, O_LOC], f32, kind="ExternalOutput").ap()

    CH = QB * 512                   # 2048 PSUM elements per chunk

    with tile.TileContext(nc) as tc:
        with (
            tc.tile_pool(name="static", bufs=1) as static,
            tc.tile_pool(name="babsp", bufs=4) as babsp,
            tc.tile_pool(name="bbp", bufs=4) as bbp,
            tc.tile_pool(name="dexpp", bufs=3) as dexpp,
            tc.tile_pool(name="et0p", bufs=8) as et0p,
            tc.tile_pool(name="et1p", bufs=2) as et1p,
            tc.tile_pool(name="dramp", bufs=1, space="DRAM") as dramp,
        ):
            # ---- stage 1 inputs load first: they gate the M' matmul --------
            # single wide transfers: the 2-3KB partition lines run at full
            # DMA rate, whereas per-chunk slices (256-400B lines) measured
            # ~9 GB/s and pushed the first matmul out to ~10us
            engs = [nc.sync, nc.scalar]
            xt_sb = static.tile([P, CC * B], fp8, tag="xt")
            t_sb = static.tile([P, CC * N_LOC], fp8, tag="t")
            th = CC * N_LOC // 2
            nc.scalar.dma_start(out=t_sb[:, 0:th], in_=T_d[:, 0:th])
            nc.sync.dma_start(out=xt_sb[:, :], in_=xT_d[:, :])
            nc.sync.dma_start(out=t_sb[:, th:], in_=T_d[:, th:])

            # rhs I-parts follow; they are not needed until stage 4
            rhs_t = []
            for h in range(2):
                rt = static.tile([K + 1, JK], fp8, tag=f"rhs{h}",
                                 name=f"rhs{h}")
                qw = JK // 2
                for s in range(2):
                    engs[s].dma_start(
                        out=rt[0:K, s * qw:(s + 1) * qw],
                        in_=rp_d[0:K, s * qw:(s + 1) * qw])
                rhs_t.append(rt)

            warm = static.tile([1, 2], f32, tag="warm")
            nc.vector.memset(warm[:, :], 0.0)
            nc.scalar.activation(out=warm[:, :], in_=warm[:, :],
                                 func=Act.Exp, scale=-1.0)
            ident = static.tile([P, P], bf16, tag="ident")
            make_identity(nc, ident[:, :])
            identf = static.tile([JBANK, JBANK], f32, tag="identf")
            make_identity(nc, identf[:, :])
            ones_col = static.tile([P, 1], bf16, tag="ones_col")
            nc.vector.memset(ones_col[:, :], 1.0)

            # +M' staged to DRAM as one flat j-major row per o, so the
            # per-o rhs row refresh is a single contiguous 12.8KB packet
            posm_d = dramp.tile([O_LOC, JK], fp8, tag="posm_d")
            m_bf = []
            m_bb = []
            with tc.tile_pool(name="mmp", bufs=2, space="PSUM") as mmp:
                for it in range(ITILES):
                    pm = mmp.tile([P, N_LOC], f32, tag="pm")
                    for cc in range(CC):
                        nc.tensor.matmul(
                            pm[:, :],
                            lhsT=xt_sb[:, cc * B + it * P: cc * B + it * P + P],
                            rhs=t_sb[:, cc * N_LOC:(cc + 1) * N_LOC],
                            start=(cc == 0), stop=(cc == CC - 1),
                        )
                    mb = static.tile([P, N_LOC], fp8, tag=f"mbf{it}",
                                     name=f"mbf{it}")
                    nc.scalar.copy(mb[:, :], pm[:, :])
                    m_bf.append(mb)
                    mbb = static.tile([P, N_LOC], bf16, tag=f"mbb{it}",
                                      name=f"mbb{it}")
                    nc.scalar.copy(mbb[:, :], mb[:, :])
                    m_bb.append(mbb)
                half = K * P
                for o in range(O_LOC):
                    for it in range(ITILES):
                        engs[(o + it) % 2].dma_start(
                            out=posm_d[o:o + 1,
                                       it * half:(it + 1) * half],
                            in_=m_bf[it][:, o * K:(o + 1) * K])

            # ---- stage 2: lhsT tiles [M'_o^T (50 rows); -ones] -------------
            # the -1 row arrives by DMA from rp row 51 (partition 50 is
            # not engine-alignable)
            lhs = []
            with tc.tile_pool(name="tpp", bufs=2, space="PSUM") as tpp:
                for o in range(O_LOC):
                    lt = static.tile([K + 1, B], fp8, tag=f"lhs{o}",
                                     name=f"lhs{o}")
                    for it in range(ITILES):
                        tp = tpp.tile([K, P], bf16, tag="tp")
                        nc.tensor.transpose(
                            tp[:, :], m_bb[it][:, o * K: o * K + K],
                            ident[:, :])
                        nc.scalar.copy(lt[0:K, it * P:(it + 1) * P], tp[:, :])
                    nc.sync.dma_start(out=lt[K:K + 1, 0:B],
                                      in_=rp_d[K + 1:K + 2, 0:B])
                    lhs.append(lt)

            # ---- stage 4: per (o, itile): diffs -> max(|p|,|m|) -> tree ----
            feat_sb = [static.tile([P, O_LOC], f32, tag=f"feat{it}",
                                   name=f"feat{it}")
                       for it in range(ITILES)]
            et0_tiles = []
            stage4 = tc.tile_pool(name="chp", bufs=1, space="PSUM")
            chp = stage4.__enter__()
            # one 8-bank PSUM ring; subtile dependency tracking lets each
            # 4-bank half recycle as soon as ITS consumers finish, instead
            # of chunk n+2 waiting on chunk n's full-tile release
            ring = chp.tile([P, 2 * CH], f32, tag="ring")
            ci = 0
            def emit_tree(ba, bb, gsl, nch):
                """Return thunks emitting the reduction tree for one group
                of nch chunks."""
                thunks = []
                ngrp = nch * BQ * JBANK
                gslc = gsl.rearrange("p (c r) -> p c r", c=nch)

                def view(ofs, width):
                    return ba[:, ofs: ofs + ngrp * width].rearrange(
                        "p (g k) -> p g k", k=width)
                bbv = bb[:, 0:ngrp * K].rearrange("p (g k) -> p g k", k=K)
                thunks.append(lambda: nc.vector.tensor_tensor(
                    out=view(0, KH), in0=bbv[:, :, 0:KH],
                    in1=bbv[:, :, KH:K], op=Alu.add))
                cur, w = 0, KH
                free = ngrp * KH
                singles = []
                while w > 1:
                    hw = w // 2
                    src_, fr = view(cur, w), free
                    rem = w - 2 * hw
                    if rem == 1:
                        singles.append(src_[:, :, w - 1:w])
                    elif rem == 2:
                        thunks.append(
                            lambda s=src_, f=fr, ww=w: nc.vector.tensor_tensor(
                                out=view(f, 1), in0=s[:, :, ww - 2:ww - 1],
                                in1=s[:, :, ww - 1:ww], op=Alu.add))
                        singles.append(view(free, 1))
                        free += ngrp
                        fr = free
                    thunks.append(
                        lambda s=src_, f=fr, h=hw: nc.vector.tensor_tensor(
                            out=view(f, h), in0=s[:, :, 0:h],
                            in1=s[:, :, h:2 * h], op=Alu.add))
                    cur = fr
                    free = fr + hw * ngrp
                    w = hw
                # final merge writes the (c, q>=ABANK, j) gsl slots
                gslb = gslc[:, :, ABANK * JBANK:JCHUNK]

                def asb(v):
                    return v.rearrange("p (c r) k -> p c (r k)", c=nch)
                for si in range(len(singles)):
                    last = si == len(singles) - 1
                    cu, fr, sv = cur, free, singles[si]
                    if last:
                        thunks.append(lambda c_=cu, s=sv: nc.vector.tensor_tensor(
                            out=gslb, in0=asb(view(c_, 1)), in1=asb(s),
                            op=Alu.add))
                    else:
                        thunks.append(
                            lambda c_=cu, f=fr, s=sv: nc.vector.tensor_tensor(
                                out=view(f, 1), in0=view(c_, 1), in1=s,
                                op=Alu.add))
                    cur = free
                    free += ngrp
                if not singles:
                    thunks.append(lambda c_=cur: nc.vector.tensor_copy(
                        out=gslb, in_=asb(view(c_, 1))))
                return thunks

            pending = []
            for o in range(O_LOC):
                rt = rhs_t[o % 2]
                nc.sync.dma_start(out=rt[K:K + 1, :],
                                  in_=posm_d[o:o + 1, :])
                for it in range(ITILES):
                    nch = NCHUNK if it == 0 else NCHUNK // 2
                    nj = nch * JCHUNK
                    dexp = dexpp.tile([P, B], f32, tag="dexp")
                    if True:
                        gsl = dexp[:, 0:nj]
                        ba = babsp.tile([P, BA_COLS], bf16, tag="ba")
                        bb = bbp.tile([P, BB_COLS], bf16, tag="bb")
                        for cc in range(nch):
                            c = cc + (0 if it == 0 else NCHUNK // 2)
                            ch = ring[:, (ci % 2) * CH:(ci % 2 + 1) * CH]
                            ci += 1
                            # B banks first: the wide ScalarE Abs can start
                            # a matmul earlier, shortening PSUM-free latency
                            for q in list(range(ABANK, QB)) + list(range(ABANK)):
                                col = (c * JCHUNK + q * JBANK) * K
                                nc.tensor.matmul(
                                    ch[:, q * 512: q * 512 + JBANK * K],
                                    lhsT=lhs[o][:, it * P:(it + 1) * P],
                                    rhs=rt[:, col: col + JBANK * K],
                                    start=True, stop=True)
                            # PSUM chunk viewed [p, q, j(8), k(50)]
                            ch4 = ch[:, :].rearrange(
                                "p (q r) -> p q r", q=QB)[
                                :, :, 0:JBANK * K].rearrange(
                                "p q (j k) -> p q j k", k=K)
                            # PSUM-freeing consumers run at high priority so
                            # the scheduler issues them ahead of queued tree
                            # adds (otherwise the PE stalls on PSUM recycle
                            # behind a tree lump in the DVE FIFO)
                            with tc.high_priority():
                                # A banks: one DVE fused |.| + k-reduce
                                # from PSUM covering all direct banks
                                nc.vector.tensor_reduce(
                                    out=gsl[:, cc * JCHUNK:
                                            cc * JCHUNK
                                            + ABANK * JBANK].rearrange(
                                        "p (q j) -> p q j", q=ABANK),
                                    in_=ch4[:, 0:ABANK, :, :],
                                    axis=mybir.AxisListType.X, op=Alu.add,
                                    apply_absolute_value=True)
                                # B banks: ScalarE |.| -> bf16 dense (j,k50)
                                nc.scalar.activation(
                                    out=bb[:, cc * BQ * JBANK * K:
                                           (cc + 1) * BQ * JBANK
                                           * K].rearrange(
                                        "p (q j k) -> p q j k",
                                        q=BQ, j=JBANK),
                                    in_=ch4[:, ABANK:QB, :, :], func=Act.Abs)
                        for t in emit_tree(ba, bb, gsl, nch):
                            t()
                    if it == 0:
                        et = et0p.tile([P, B], bf16, tag="et0",
                                       name=f"et0_{o}")
                        et0_tiles.append(et)
                    else:
                        et = et1p.tile([P, B // 2], bf16, tag="et1")
                    # exp(-l1) + fused j-sum into the feature column
                    nc.scalar.activation(out=et[:, :], in_=dexp[:, 0:nj],
                                         func=Act.Exp, scale=-1.0,
                                         accum_out=feat_sb[it][:, o:o + 1])
            stage4.__exit__(None, None, None)

            # ---- stage 5: mirrored contribution for itile 1 ----------------
            # colsum_o[j] = sum_{i in it0} exp(-D[i, j]) for j in [128, 256)
            cs_sb = static.tile([JBANK, P], f32, tag="cs_sb")
            with tc.tile_pool(name="csp", bufs=2, space="PSUM") as csp:
                for o in range(O_LOC):
                    cs = csp.tile([1, P], f32, tag="cs")
                    nc.tensor.matmul(cs[:, :], lhsT=ones_col[:, :],
                                     rhs=et0_tiles[o][:, P:B],
                                     start=True, stop=True)
                    cs_row = babsp.tile([1, P], f32, tag="cs_row")
                    nc.scalar.copy(cs_row[:, :], cs[:, :])
                    nc.sync.dma_start(out=cs_sb[o:o + 1, :], in_=cs_row[:, :])
                ct = csp.tile([P, JBANK], f32, tag="ct")
                nc.tensor.transpose(ct[:, :], cs_sb[:, :], identf[:, :])
                nc.vector.tensor_tensor(out=feat_sb[1][:, :],
                                        in0=feat_sb[1][:, :],
                                        in1=ct[:, :], op=Alu.add)

            for it in range(ITILES):
                nc.vector.tensor_scalar(
                    out=feat_sb[it][:, :], in0=feat_sb[it][:, :],
                    scalar1=1.0, scalar2=None, op0=Alu.subtract)
                nc.sync.dma_start(out=feat_d[it * P:(it + 1) * P, :],
                                  in_=feat_sb[it][:, :])

    nc.compile()
    return nc


def _get_program():
    if "nc" not in _cache:
        _cache["nc"] = _build_program()
    return _cache["nc"]


def prepare_in_maps(x, T):
    """Host-side sharding: transpose/cast x, pair-transform + slice T."""
    f8 = ml_dtypes.float8_e4m3fn
    xf = np.asarray(x, dtype=np.float32)
    # xT host layout [128, CC*B]: partition p holds x^T rows p+128*cc
    xT = np.ascontiguousarray(xf.T).reshape(CC, P, B).transpose(1, 0, 2)
    xT = np.ascontiguousarray(xT.reshape(P, CC * B)).astype(f8)
    Tp = np.asarray(T, dtype=np.float32).reshape(IN_FEATURES, O_TOTAL, K)
    in_maps = []
    rp = np.zeros((K + 2, JK), dtype=f8)
    kk = np.arange(K)
    for j in range(B):
        rp[kk, j * K + kk] = 1.0
    rp[K + 1, :] = -1.0
    for c in range(N_CORES):
        Tl = Tp[:, c * O_LOC:(c + 1) * O_LOC, :].reshape(IN_FEATURES, N_LOC)
        # host layout [128, CC*N_LOC]: partition p holds rows p+128*cc
        Tl = np.ascontiguousarray(Tl).reshape(CC, P, N_LOC).transpose(1, 0, 2)
        Tl = np.ascontiguousarray(Tl.reshape(P, CC * N_LOC)).astype(f8)
        in_maps.append({"xT": xT, "Tl": Tl, "rp": rp})
    return in_maps


def run_cores(in_maps, trace=False, tmpdir=None):
    from concourse import bass_utils
    nc = _get_program()
    return bass_utils.run_bass_kernel_spmd(
        nc, in_maps, core_ids=list(range(N_CORES)), trace=trace, tmpdir=tmpdir)


def kernel(x, T):
    x = np.asarray(x, dtype=np.float32)
    res = run_cores(prepare_in_maps(x, T))
    feat = np.concatenate(
        [res.results[c]["feat"].astype(np.float32) for c in range(N_CORES)],
        axis=1)
    return np.concatenate([x, feat], axis=1)
